# revision 2
# baseline (speedup 1.0000x reference)
"""Butterfly rotation (10 stages, DIM=1024) on 8 Trainium2 NeuronCores.

Math: each row x[n, :] undergoes 10 butterfly rotation stages; the whole
transform is linear.  Stages 0..7 (strides 1..128) only mix elements within
256-wide blocks, so their composite is block-diagonal with four dense
256x256 blocks (precomputed on host from `angles`).  Stages 8 and 9
(strides 256/512) pair whole 128-dim chunks and are applied on-chip as
per-element rotations with per-partition cos/sin scalars.

The 2e-2 relative-error budget admits fp16 end-to-end: fp16 weights and
inputs quadruple TensorE throughput vs fp32 (1 moving column/cycle) and
halve HBM traffic (the binding roofline: ~358 GB/s/core, 16 MiB in +
16 MiB out ~ 94 us).  Measured pipeline error ~1e-3.

Device layout (per core, rows sharded 8192/core; pure data parallelism):
  - host packs each core's shard dim-major in fp16:
    xin[g, p, c*1024 + r] = x[g*1024 + r, c*128 + p]   (g: 8 row-groups,
    c: 8 dim-chunks of 128, p: dim-within-chunk, r: row-within-group)
  - PE: per chunk, 4 fp16 matmuls [K=128, M=128, N=512] (2 accumulating
    per 512-row half) -> PSUM fp32 [128, 1024].
  - ACT evicts PSUM -> SBUF fp16 (the only fp32-rate sweep).
  - stage 8 (chunk pairs (0,2),(1,3),(4,6),(5,7)): POOL pre-scales the
    cross terms (tensor_scalar, per-partition scalar), DVE combines with
    scalar_tensor_tensor at the 16-bit 2x rate.
  - stage 9 (pairs (cg, cg+4)): two pairs via 3-shear rotations on DVE
    (no prescale needed; u += a*v, v += s*u, u += a*v with
    a = -tan(th'/2) bounded by extracting sign(cos th9) into stage-8's
    scalars), two pairs via POOL-prescale + DVE stt to balance engines.
  - loads ride the SP HWDGE ring, stores the ACT ring; ~1 MiB each.
  - host inverse-permutes and upcasts the fp16 output.
"""

import os
import sys

sys.path.insert(0, "/opt/trn_rl_repo")

# run_bass_kernel_spmd would try to import the (absent) axon NTFF hook if
# BASS_TRACE is set in the environment.
os.environ["BASS_NEVER_TRACE"] = "1"

import numpy as np

DIM = 1024
STAGES = 10
N_CORES = 8
ROWS_PER_CORE = 8192
GROUP_ROWS = 1024
N_GROUPS = ROWS_PER_CORE // GROUP_ROWS  # 8

# stage-8 chunk pairs in PE-emission order (pi = trig column index is the
# position in *this* list's sorted-by-u order: (0,2)->0, (1,3)->1,
# (4,6)->2, (5,7)->3); order (0,2),(4,6) first so stage-9 pairs (0,4) and
# (2,6) can start after two stage-8 pairs.
S8_PAIRS = [(0, 2, 0), (4, 6, 2), (1, 3, 1), (5, 7, 3)]
# stage-9 pair cg -> mode ('shear' = 3 stt on DVE, 'stt' = POOL prescale +
# DVE stt); after S8 pairs (0,2),(4,6): cg 0 and 2; after (1,3),(5,7): 1, 3
S9_SCHED = [(0, "shear"), (2, "stt"), (1, "shear"), (3, "stt")]


def _stage_idx(dim, stage):
    stride = 2**stage
    idx_i = np.arange(dim).reshape(-1, 2 * stride)[:, :stride].ravel()
    idx_j = idx_i + stride
    return idx_i, idx_j


def _butterfly_apply(v, angles, stages):
    """Apply butterfly stages to rows of v (float64, in place) and return v."""
    for s in stages:
        idx_i, idx_j = _stage_idx(v.shape[1], s)
        c = np.cos(angles[s].astype(np.float64))
        sn = np.sin(angles[s].astype(np.float64))
        vi = v[:, idx_i].copy()
        vj = v[:, idx_j].copy()
        v[:, idx_i] = c * vi - sn * vj
        v[:, idx_j] = sn * vi + c * vj
    return v


def _host_tables(angles):
    """wts[k, b, m] fp16, b = c*2 + t: lhsT block for output chunk c (0..7),
    input chunk ci = 2*(c//2) + t.

    trig[p, :] fp32 per-partition scalar columns:
      0:4   -sa*s8 per S8 pair   4:8  sb*s8      8:12  sa*c8
      12:16  sb*c8               (sa/sb = stage-9 sign of the pair's chunks)
      16:20 a9 = -tan(th9'/2)    20:24 sin th9'  24:28 cos th9'
      28:32 -sin th9'            (th9' = th9 sign-normalized, cos >= 0)
    """
    mb = _butterfly_apply(np.eye(DIM, dtype=np.float64), angles, range(STAGES - 2))
    wts = np.empty((128, 16, 128), dtype=np.float16)
    for c in range(8):
        q = c // 2
        blk = mb[q * 256 : (q + 1) * 256, q * 256 : (q + 1) * 256]
        jl = (c % 2) * 128
        for t in range(2):
            wts[:, c * 2 + t, :] = blk[t * 128 : (t + 1) * 128, jl : jl + 128].astype(
                np.float16
            )
    # off-block-diagonal must vanish for stages 0..7
    mask = np.ones((DIM, DIM), dtype=bool)
    for q in range(4):
        mask[q * 256 : (q + 1) * 256, q * 256 : (q + 1) * 256] = False
    assert abs(mb[mask]).max() == 0.0

    th8 = angles[8].astype(np.float64)
    th9 = angles[9].astype(np.float64)
    # stage-9 sign extraction: R(th9) = sg * R(th9'), cos th9' >= 0
    sg9, cth, sth, a9 = [], [], [], []
    for cg in range(4):
        th = th9[cg * 128 : (cg + 1) * 128]
        sg = np.where(np.cos(th) >= 0, 1.0, -1.0)
        c9, s9 = sg * np.cos(th), sg * np.sin(th)
        thp = np.arctan2(s9, c9)
        sg9.append(sg)
        cth.append(c9)
        sth.append(s9)
        a9.append(-np.tan(thp / 2))
        assert np.abs(a9[-1]).max() <= 1.0 + 1e-12

    trig = np.empty((128, 32), dtype=np.float32)
    for u, v, pi in S8_PAIRS:
        th = th8[pi * 128 : (pi + 1) * 128]
        c8, s8 = np.cos(th), np.sin(th)
        sa, sb = sg9[u % 4], sg9[v % 4]
        trig[:, 0 + pi] = -sa * s8
        trig[:, 4 + pi] = sb * s8
        trig[:, 8 + pi] = sa * c8
        trig[:, 12 + pi] = sb * c8
    for cg in range(4):
        trig[:, 16 + cg] = a9[cg]
        trig[:, 20 + cg] = sth[cg]
        trig[:, 24 + cg] = cth[cg]
        trig[:, 28 + cg] = -sth[cg]
    return wts, trig


def _pack_x(x_core, n_groups=N_GROUPS):
    # [G*1024, 1024] -> [G, 128, 8192] with xin[g, p, c*1024+r] = x[g*1024+r, c*128+p]
    g = x_core.reshape(n_groups, GROUP_ROWS, 8, 128)
    return np.ascontiguousarray(
        g.transpose(0, 3, 2, 1).reshape(n_groups, 128, 8 * GROUP_ROWS)
    )


def _unpack_y(y_packed, n_groups=N_GROUPS):
    # yout[g, p, c*1024 + r] = y[g*1024 + r, c*128 + p]
    g = y_packed.reshape(n_groups, 128, 8, GROUP_ROWS)
    return np.ascontiguousarray(
        g.transpose(0, 3, 2, 1).reshape(n_groups * GROUP_ROWS, DIM)
    )


def _patch_tile_drain():
    """Workaround: this walrus build cannot encode semaphore waits on a
    sequencer Drain/NoOp with >1 wait ("Too many sync wait commands").
    Re-emit the TileContext tail waits as one nop per semaphore."""
    from concourse import mybir, tile
    from concourse.vector_clock import ScopedClock

    if getattr(tile.TileContext, "_drain_patched", False):
        return

    def _drain_and_barrier(self, tick_clock, wait_clock):
        nop_inst = self.nc.sync.nop(nofuse=True)
        wait_clock.add_sem_waits(
            nop_inst.ins, ScopedClock({None: tick_clock.global_clock})
        )
        si = nop_inst.ins.sync_info
        if si is not None and si.on_wait and len(si.on_wait) > 1:
            extra = si.on_wait[1:]
            si.on_wait = si.on_wait[:1]
            for w in extra:
                extra_nop = self.nc.sync.nop(nofuse=True)
                esi = extra_nop.ins.sync_info
                if esi is None:
                    extra_nop.ins.sync_info = mybir.SyncInfo(on_wait=[w], on_update=[])
                else:
                    esi.on_wait = list(esi.on_wait or []) + [w]
        self.nc.sync.drain()
        self.nc.all_engine_barrier()
        assert self.sems is not None
        popped = self.nc._tile_sem_poison_stack.pop()
        assert popped is self._sem_poison
        self.nc.clear_and_free_semaphores(list(self.sems.allocated().values()))
        self.nc.all_engine_barrier()

    tile.TileContext._drain_and_barrier = _drain_and_barrier
    tile.TileContext._drain_patched = True


def _split_multi_waits(nc, limit=1):
    """This walrus build encodes at most `limit` semaphore wait(s) per
    instruction ("Too many sync wait commands").  Hoist excess waits onto
    same-engine NoOps inserted immediately before the instruction."""
    from concourse import mybir

    counter = [0]

    def fresh_nop(engine, waits):
        counter[0] += 1
        nop = mybir.InstNoOp(
            name=f"waitsplit-{counter[0]}",
            engine=engine,
            ins=[],
            outs=[],
            bass_nofuse=True,
            sync_info=mybir.SyncInfo(on_wait=list(waits), on_update=[]),
        )
        nc.register_instruction(nop, overwrite=True)
        return nop

    for fn in nc.m.functions:
        for bb in fn.blocks:
            changed = False
            new = []
            for inst in bb.instructions:
                si = getattr(inst, "sync_info", None)
                if si is not None and si.on_wait and len(si.on_wait) > limit:
                    extra = si.on_wait[: len(si.on_wait) - limit]
                    si.on_wait = si.on_wait[len(si.on_wait) - limit :]
                    for k in range(0, len(extra), limit):
                        new.append(fresh_nop(inst.engine, extra[k : k + limit]))
                    changed = True
                new.append(inst)
            if changed:
                bb.instructions = new
    return nc


def build_bass(n_groups=N_GROUPS, reps=1):
    """Build the Bass module for one core processing n_groups row-groups.
    reps>1 repeats the whole pipeline in-NEFF (for timing calibration)."""
    _patch_tile_drain()
    from concourse import bass, mybir, tile

    f16 = mybir.dt.float16
    f32 = mybir.dt.float32
    nc = bass.Bass("TRN2", target_bir_lowering=False, debug=False)
    xin = nc.dram_tensor("xin", [n_groups, 128, 8192], f16, kind="ExternalInput")
    wts = nc.dram_tensor("wts", [128, 16, 128], f16, kind="ExternalInput")
    trig = nc.dram_tensor("trig", [128, 32], f32, kind="ExternalInput")
    yout = nc.dram_tensor("yout", [n_groups, 128, 8192], f16, kind="ExternalOutput")

    mult = mybir.AluOpType.mult
    add = mybir.AluOpType.add

    with tile.TileContext(nc) as tc:
        with (
            tc.tile_pool(name="wp", bufs=1) as wp,
            tc.tile_pool(name="xp", bufs=3) as xp,
            tc.tile_pool(name="yp", bufs=2) as yp,
            tc.tile_pool(name="ep", bufs=6) as ep,   # PSUM evictions (fp16)
            tc.tile_pool(name="e9", bufs=10) as e9p,  # stage-8 outputs
            tc.tile_pool(name="pp", bufs=4) as pp,   # POOL prescales
            tc.tile_pool(name="t9", bufs=3) as t9p,  # shear temps
            tc.tile_pool(name="ps", bufs=4, space="PSUM") as psp,
        ):
            wt = wp.tile([128, 16, 128], f16)
            nc.sync.dma_start(wt[:], wts.ap()[:])
            tg = wp.tile([128, 32], f32)
            nc.sync.dma_start(tg[:], trig.ap()[:])

            for g in [g for _ in range(reps) for g in range(n_groups)]:
                xt = xp.tile([128, 8192], f16)
                nc.sync.dma_start(xt[:, 0:4096], xin.ap()[g][:, 0:4096])
                nc.sync.dma_start(xt[:, 4096:8192], xin.ap()[g][:, 4096:8192])
                yt = yp.tile([128, 8192], f16)
                E9 = [None] * 8
                for pair_n, (u, v, pi) in enumerate(S8_PAIRS):
                    ev = {}
                    for c in (u, v):
                        p = psp.tile([128, 1024], f32, tag="ps")
                        for t in range(2):
                            ci = 2 * (c // 2) + t
                            for h in range(2):
                                nc.tensor.matmul(
                                    p[:, h * 512 : (h + 1) * 512],
                                    wt[:, c * 2 + t, :],
                                    xt[:, ci * 1024 + h * 512 : ci * 1024 + (h + 1) * 512],
                                    start=(t == 0),
                                    stop=(t == 1),
                                )
                        e = ep.tile([128, 1024], f16, tag="e")
                        nc.scalar.copy(e[:], p[:])
                        ev[c] = e
                    # stage 8: A = sa*c8.Eu + (-sa*s8).Ev ; B = sb*c8.Ev + sb*s8.Eu
                    p1 = pp.tile([128, 1024], f16, tag="p")
                    nc.gpsimd.tensor_scalar_mul(p1[:], ev[v][:], tg[:, 0 + pi : 1 + pi])
                    a = e9p.tile([128, 1024], f16, tag="e9")
                    nc.vector.scalar_tensor_tensor(
                        a[:], ev[u][:], tg[:, 8 + pi : 9 + pi], p1[:], mult, add
                    )
                    p2 = pp.tile([128, 1024], f16, tag="p")
                    nc.gpsimd.tensor_scalar_mul(p2[:], ev[u][:], tg[:, 4 + pi : 5 + pi])
                    b = e9p.tile([128, 1024], f16, tag="e9")
                    nc.vector.scalar_tensor_tensor(
                        b[:], ev[v][:], tg[:, 12 + pi : 13 + pi], p2[:], mult, add
                    )
                    E9[u], E9[v] = a, b
                    if pair_n % 2 == 0:
                        continue
                    # two stage-9 pairs become ready after each odd S8 pair
                    for cg, mode in S9_SCHED[pair_n - 1 : pair_n + 1]:
                        lo, hi = E9[cg], E9[cg + 4]
                        ylo = yt[:, cg * 1024 : (cg + 1) * 1024]
                        yhi = yt[:, (cg + 4) * 1024 : (cg + 5) * 1024]
                        if mode == "shear":
                            u1 = t9p.tile([128, 1024], f16, tag="t9")
                            nc.vector.scalar_tensor_tensor(
                                u1[:], hi[:], tg[:, 16 + cg : 17 + cg], lo[:], mult, add
                            )
                            nc.vector.scalar_tensor_tensor(
                                yhi, u1[:], tg[:, 20 + cg : 21 + cg], hi[:], mult, add
                            )
                            nc.vector.scalar_tensor_tensor(
                                ylo, yhi, tg[:, 16 + cg : 17 + cg], u1[:], mult, add
                            )
                        else:
                            q1 = pp.tile([128, 1024], f16, tag="p")
                            nc.gpsimd.tensor_scalar_mul(
                                q1[:], hi[:], tg[:, 28 + cg : 29 + cg]
                            )
                            nc.vector.scalar_tensor_tensor(
                                ylo, lo[:], tg[:, 24 + cg : 25 + cg], q1[:], mult, add
                            )
                            q2 = pp.tile([128, 1024], f16, tag="p")
                            nc.gpsimd.tensor_scalar_mul(
                                q2[:], lo[:], tg[:, 20 + cg : 21 + cg]
                            )
                            nc.vector.scalar_tensor_tensor(
                                yhi, hi[:], tg[:, 24 + cg : 25 + cg], q2[:], mult, add
                            )
                nc.scalar.dma_start(yout.ap()[g][:, 0:4096], yt[:, 0:4096])
                nc.scalar.dma_start(yout.ap()[g][:, 4096:8192], yt[:, 4096:8192])
    _split_multi_waits(nc)
    return nc


_CACHE = {}


def _get_nc(n_groups=N_GROUPS):
    if n_groups not in _CACHE:
        _CACHE[n_groups] = build_bass(n_groups)
    return _CACHE[n_groups]


def make_in_maps(x, angles):
    """Pack full inputs into per-core in_maps (list of dicts)."""
    x = np.asarray(x, dtype=np.float32)
    angles = np.asarray(angles, dtype=np.float32)
    wts, trig = _host_tables(angles)
    flat = x.reshape(-1, DIM).astype(np.float16)
    in_maps = []
    for k in range(N_CORES):
        shard = flat[k * ROWS_PER_CORE : (k + 1) * ROWS_PER_CORE]
        in_maps.append({"xin": _pack_x(shard), "wts": wts, "trig": trig})
    return in_maps


def kernel(x, angles):
    from concourse.bass_utils import run_bass_kernel_spmd

    x = np.asarray(x)
    orig_shape = x.shape
    in_maps = make_in_maps(x, angles)
    nc = _get_nc()
    res = run_bass_kernel_spmd(nc, in_maps, core_ids=list(range(N_CORES)))
    parts = [_unpack_y(res.results[k]["yout"]) for k in range(N_CORES)]
    out = np.concatenate(parts, axis=0).reshape(orig_shape)
    return out.astype(np.float32)


# revision 3
# speedup vs baseline: 8.5799x; 8.5799x over previous
"""Butterfly rotation (10 stages, DIM=1024) on 8 Trainium2 NeuronCores.

Math: the 10-stage butterfly transform is linear.  Stages 0..8 (strides
1..256) mix only within 512-wide blocks: their composite is block-diagonal
with two dense 512x512 blocks, applied on the PE as fp16 matmuls (the
2e-2 error budget admits fp16 end-to-end; measured pipeline error ~1e-3).
Stage 9 (stride 512) pairs chunk cg with cg+4 (128-dim chunks) and is an
elementwise per-partition rotation.

Measured per-[128,1024]-pass engine menu (this deployment): ACT copy from
PSUM 731 ns / with [128,1] scale 1025 ns; DVE fp16 stt 815 ns / TT 635 ns
(SBUF; PSUM-sourced DVE is 1.8 us - avoided); GPSIMD ~8-15 us (unusable);
PE fp16 [128,128,512] matmul 98 ns; DMA floor ~41 us.  PSUM eviction must
therefore ride ACT, stage-9 combines ride DVE in SBUF, and the PE (which
has slack under the DMA roofline) absorbs some chunk pairs entirely as
dense 1024-wide rows of the full 10-stage composite.

Per stage-9 pair (cg, cg+4), three implementations (mix is tuned to
balance engines):
  dense:  PE computes the final outputs via dense rows (16 MM/chunk);
          ACT evicts straight into the output tile.  No DVE work.
  shear:  PE computes stage-0..8 chunks z (8 MM/chunk, with the stage-9
          sign sg = sign(cos th9) folded into the weight rows so the
          residual rotation angle th9' has cos >= 0); ACT evicts z to
          fp16; DVE applies the rotation as 3 shears
          (u += a.v; v += s.u; u += a.v with a = -tan(th9'/2), |a| <= 1).
  acth:   like shear but ACT evicts 4 scaled copies (c9'.z_lo, -s9'.z_hi,
          s9'.z_lo, c9'.z_hi) and DVE just adds pairs (2 TT) - shifts
          work from DVE to ACT.

Device layout (per core, rows sharded 8192/core; pure data parallelism):
  host packs each core's shard dim-major fp16:
  xin[g, p, c*1024 + r] = x[g*1024 + r, c*128 + p]  (g: 8 row-groups of
  1024 rows, c: 8 dim-chunks of 128, p: dim-within-chunk).  Loads ride
  the SP HWDGE ring, stores the ACT ring, ~1 MiB per transfer.  Host
  inverse-permutes and upcasts the fp16 output.
"""

import os
import sys

sys.path.insert(0, "/opt/trn_rl_repo")

# run_bass_kernel_spmd would try to import the (absent) axon NTFF hook if
# BASS_TRACE is set in the environment.
os.environ["BASS_NEVER_TRACE"] = "1"

import numpy as np

DIM = 1024
STAGES = 10
N_CORES = 8
ROWS_PER_CORE = 8192
GROUP_ROWS = 1024
N_GROUPS = ROWS_PER_CORE // GROUP_ROWS  # 8

# stage-9 pair cg -> implementation; tuned so PE/ACT/DVE land together
PAIR_MODES = ["dense", "shear", "shear", "shear"]


def _stage_idx(dim, stage):
    stride = 2**stage
    idx_i = np.arange(dim).reshape(-1, 2 * stride)[:, :stride].ravel()
    idx_j = idx_i + stride
    return idx_i, idx_j


def _butterfly_apply(v, angles, stages):
    """Apply butterfly stages to rows of v (float64, in place) and return v."""
    for s in stages:
        idx_i, idx_j = _stage_idx(v.shape[1], s)
        c = np.cos(angles[s].astype(np.float64))
        sn = np.sin(angles[s].astype(np.float64))
        vi = v[:, idx_i].copy()
        vj = v[:, idx_j].copy()
        v[:, idx_i] = c * vi - sn * vj
        v[:, idx_j] = sn * vi + c * vj
    return v


def _host_tables(angles):
    """w9[k, c*4+t, m] fp16: lhsT for stage-0..8 output chunk c from input
    chunk ci = 4*(c//4) + t, rows pre-scaled by sg9[c % 4] (stage-9 sign).
    wd[k, c*8+t, m] fp16: dense full-composite lhsT, output chunk c from
    input chunk t.
    trig[p, 0..3]=a9[cg], [4..7]=sin th9', [8..11]=cos th9', [12..15]=-sin.
    """
    # _butterfly_apply on eye gives mb[i, j] = M[j, i] (M maps in->out),
    # so lhsT[k, m] = M[c*128+m, ci*128+k] = mb[ci*128+k, c*128+m].
    mb9 = _butterfly_apply(np.eye(DIM, dtype=np.float64), angles, range(9))
    mbA = _butterfly_apply(mb9.copy(), angles, [9])
    mask = np.ones((DIM, DIM), dtype=bool)
    for h in range(2):
        mask[h * 512 : (h + 1) * 512, h * 512 : (h + 1) * 512] = False
    assert abs(mb9[mask]).max() == 0.0

    th9 = angles[9].astype(np.float64)
    sg9, cth, sth, a9 = [], [], [], []
    for cg in range(4):
        th = th9[cg * 128 : (cg + 1) * 128]
        sg = np.where(np.cos(th) >= 0, 1.0, -1.0)
        c9, s9 = sg * np.cos(th), sg * np.sin(th)
        sg9.append(sg)
        cth.append(c9)
        sth.append(s9)
        a9.append(-np.tan(np.arctan2(s9, c9) / 2))
        assert np.abs(a9[-1]).max() <= 1.0 + 1e-12

    w9 = np.empty((128, 32, 128), dtype=np.float16)
    for c in range(8):
        for t in range(4):
            ci = 4 * (c // 4) + t
            blk = mb9[ci * 128 : (ci + 1) * 128, c * 128 : (c + 1) * 128]
            w9[:, c * 4 + t, :] = (blk * sg9[c % 4][None, :]).astype(np.float16)
    wd = np.empty((128, 64, 128), dtype=np.float16)
    for c in range(8):
        for t in range(8):
            wd[:, c * 8 + t, :] = mbA[
                t * 128 : (t + 1) * 128, c * 128 : (c + 1) * 128
            ].astype(np.float16)

    trig = np.empty((128, 16), dtype=np.float32)
    for cg in range(4):
        trig[:, 0 + cg] = a9[cg]
        trig[:, 4 + cg] = sth[cg]
        trig[:, 8 + cg] = cth[cg]
        trig[:, 12 + cg] = -sth[cg]
    return w9, wd, trig


def _pack_x(x_core, n_groups=N_GROUPS):
    # [G*1024, 1024] -> [G, 128, 8192] with xin[g, p, c*1024+r] = x[g*1024+r, c*128+p]
    g = x_core.reshape(n_groups, GROUP_ROWS, 8, 128)
    return np.ascontiguousarray(
        g.transpose(0, 3, 2, 1).reshape(n_groups, 128, 8 * GROUP_ROWS)
    )


def _unpack_y(y_packed, n_groups=N_GROUPS):
    # yout[g, p, c*1024 + r] = y[g*1024 + r, c*128 + p]
    g = y_packed.reshape(n_groups, 128, 8, GROUP_ROWS)
    return np.ascontiguousarray(
        g.transpose(0, 3, 2, 1).reshape(n_groups * GROUP_ROWS, DIM)
    )


def _patch_tile_drain():
    """Workaround: this walrus build cannot encode semaphore waits on a
    sequencer Drain/NoOp with >1 wait ("Too many sync wait commands").
    Re-emit the TileContext tail waits as one nop per semaphore."""
    from concourse import mybir, tile
    from concourse.vector_clock import ScopedClock

    if getattr(tile.TileContext, "_drain_patched", False):
        return

    def _drain_and_barrier(self, tick_clock, wait_clock):
        nop_inst = self.nc.sync.nop(nofuse=True)
        wait_clock.add_sem_waits(
            nop_inst.ins, ScopedClock({None: tick_clock.global_clock})
        )
        si = nop_inst.ins.sync_info
        if si is not None and si.on_wait and len(si.on_wait) > 1:
            extra = si.on_wait[1:]
            si.on_wait = si.on_wait[:1]
            for w in extra:
                extra_nop = self.nc.sync.nop(nofuse=True)
                esi = extra_nop.ins.sync_info
                if esi is None:
                    extra_nop.ins.sync_info = mybir.SyncInfo(on_wait=[w], on_update=[])
                else:
                    esi.on_wait = list(esi.on_wait or []) + [w]
        self.nc.sync.drain()
        self.nc.all_engine_barrier()
        assert self.sems is not None
        popped = self.nc._tile_sem_poison_stack.pop()
        assert popped is self._sem_poison
        self.nc.clear_and_free_semaphores(list(self.sems.allocated().values()))
        self.nc.all_engine_barrier()

    tile.TileContext._drain_and_barrier = _drain_and_barrier
    tile.TileContext._drain_patched = True


def _split_multi_waits(nc, limit=1):
    """This walrus build encodes at most `limit` semaphore wait(s) per
    instruction ("Too many sync wait commands").  Hoist excess waits onto
    same-engine NoOps inserted immediately before the instruction."""
    from concourse import mybir

    counter = [0]

    def fresh_nop(engine, waits):
        counter[0] += 1
        nop = mybir.InstNoOp(
            name=f"waitsplit-{counter[0]}",
            engine=engine,
            ins=[],
            outs=[],
            bass_nofuse=True,
            sync_info=mybir.SyncInfo(on_wait=list(waits), on_update=[]),
        )
        nc.register_instruction(nop, overwrite=True)
        return nop

    for fn in nc.m.functions:
        for bb in fn.blocks:
            changed = False
            new = []
            for inst in bb.instructions:
                si = getattr(inst, "sync_info", None)
                if si is not None and si.on_wait and len(si.on_wait) > limit:
                    extra = si.on_wait[: len(si.on_wait) - limit]
                    si.on_wait = si.on_wait[len(si.on_wait) - limit :]
                    for k in range(0, len(extra), limit):
                        new.append(fresh_nop(inst.engine, extra[k : k + limit]))
                    changed = True
                new.append(inst)
            if changed:
                bb.instructions = new
    return nc


def build_bass(n_groups=N_GROUPS, reps=1, pair_modes=None):
    """Build the Bass module for one core processing n_groups row-groups.
    reps>1 repeats the whole pipeline in-NEFF (for timing calibration)."""
    _patch_tile_drain()
    from concourse import bass, mybir, tile

    pair_modes = pair_modes or PAIR_MODES
    f16 = mybir.dt.float16
    f32 = mybir.dt.float32
    nc = bass.Bass("TRN2", target_bir_lowering=False, debug=False)
    xin = nc.dram_tensor("xin", [n_groups, 128, 8192], f16, kind="ExternalInput")
    w9d = nc.dram_tensor("w9", [128, 32, 128], f16, kind="ExternalInput")
    wdd = nc.dram_tensor("wd", [128, 64, 128], f16, kind="ExternalInput")
    trig = nc.dram_tensor("trig", [128, 16], f32, kind="ExternalInput")
    yout = nc.dram_tensor("yout", [n_groups, 128, 8192], f16, kind="ExternalOutput")

    mult = mybir.AluOpType.mult
    add = mybir.AluOpType.add
    copy_fn = mybir.ActivationFunctionType.Copy

    def mm_chunk(psum, wtile, base, nw, xt, c):
        """Accumulate chunk c's output into psum from nw input chunks."""
        for t in range(nw):
            ci = (c // 4) * 4 + t if nw == 4 else t
            for h in range(2):
                nc.tensor.matmul(
                    psum[:, h * 512 : (h + 1) * 512],
                    wtile[:, base + t, :],
                    xt[:, ci * 1024 + h * 512 : ci * 1024 + (h + 1) * 512],
                    start=(t == 0),
                    stop=(t == nw - 1),
                )

    with tile.TileContext(nc) as tc:
        with (
            tc.tile_pool(name="wp", bufs=1) as wp,
            tc.tile_pool(name="xp", bufs=3) as xp,
            tc.tile_pool(name="yp", bufs=2) as yp,
            tc.tile_pool(name="ep", bufs=8) as ep,
            tc.tile_pool(name="tp", bufs=3) as tp,
            tc.tile_pool(name="ps", bufs=4, space="PSUM") as psp,
        ):
            w9 = wp.tile([128, 32, 128], f16)
            nc.sync.dma_start(w9[:], w9d.ap()[:])
            wd = wp.tile([128, 64, 128], f16)
            nc.sync.dma_start(wd[:], wdd.ap()[:])
            tg = wp.tile([128, 16], f32)
            nc.sync.dma_start(tg[:], trig.ap()[:])

            for g in [g for _ in range(reps) for g in range(n_groups)]:
                xt = xp.tile([128, 8192], f16)
                nc.sync.dma_start(xt[:, 0:4096], xin.ap()[g][:, 0:4096])
                nc.sync.dma_start(xt[:, 4096:8192], xin.ap()[g][:, 4096:8192])
                yt = yp.tile([128, 8192], f16)
                for cg in range(4):
                    mode = pair_modes[cg]
                    lo, hi = cg, cg + 4
                    ylo = yt[:, lo * 1024 : (lo + 1) * 1024]
                    yhi = yt[:, hi * 1024 : (hi + 1) * 1024]
                    plo = psp.tile([128, 1024], f32, tag="ps")
                    phi = psp.tile([128, 1024], f32, tag="ps")
                    if mode == "dense":
                        mm_chunk(plo, wd, lo * 8, 8, xt, lo)
                        mm_chunk(phi, wd, hi * 8, 8, xt, hi)
                        nc.scalar.copy(ylo, plo[:])
                        nc.scalar.copy(yhi, phi[:])
                        continue
                    mm_chunk(plo, w9, lo * 4, 4, xt, lo)
                    mm_chunk(phi, w9, hi * 4, 4, xt, hi)
                    if mode == "shear":
                        el = ep.tile([128, 1024], f16, tag="e")
                        nc.scalar.copy(el[:], plo[:])
                        eh = ep.tile([128, 1024], f16, tag="e")
                        nc.scalar.copy(eh[:], phi[:])
                        u1 = tp.tile([128, 1024], f16, tag="t")
                        nc.vector.scalar_tensor_tensor(
                            u1[:], eh[:], tg[:, 0 + cg : 1 + cg], el[:], mult, add
                        )
                        nc.vector.scalar_tensor_tensor(
                            yhi, u1[:], tg[:, 4 + cg : 5 + cg], eh[:], mult, add
                        )
                        nc.vector.scalar_tensor_tensor(
                            ylo, yhi, tg[:, 0 + cg : 1 + cg], u1[:], mult, add
                        )
                    else:  # acth
                        e1 = ep.tile([128, 1024], f16, tag="e")
                        nc.scalar.activation(e1[:], plo[:], copy_fn, scale=tg[:, 8 + cg : 9 + cg])
                        p1 = ep.tile([128, 1024], f16, tag="e")
                        nc.scalar.activation(p1[:], phi[:], copy_fn, scale=tg[:, 12 + cg : 13 + cg])
                        nc.vector.tensor_tensor(ylo, e1[:], p1[:], add)
                        e2 = ep.tile([128, 1024], f16, tag="e")
                        nc.scalar.activation(e2[:], plo[:], copy_fn, scale=tg[:, 4 + cg : 5 + cg])
                        p2 = ep.tile([128, 1024], f16, tag="e")
                        nc.scalar.activation(p2[:], phi[:], copy_fn, scale=tg[:, 8 + cg : 9 + cg])
                        nc.vector.tensor_tensor(yhi, e2[:], p2[:], add)
                nc.scalar.dma_start(yout.ap()[g][:, 0:4096], yt[:, 0:4096])
                nc.scalar.dma_start(yout.ap()[g][:, 4096:8192], yt[:, 4096:8192])
    _split_multi_waits(nc)
    return nc


_CACHE = {}


def _get_nc(n_groups=N_GROUPS):
    if n_groups not in _CACHE:
        _CACHE[n_groups] = build_bass(n_groups)
    return _CACHE[n_groups]


def make_in_maps(x, angles):
    """Pack full inputs into per-core in_maps (list of dicts)."""
    x = np.asarray(x, dtype=np.float32)
    angles = np.asarray(angles, dtype=np.float32)
    w9, wd, trig = _host_tables(angles)
    flat = x.reshape(-1, DIM).astype(np.float16)
    in_maps = []
    for k in range(N_CORES):
        shard = flat[k * ROWS_PER_CORE : (k + 1) * ROWS_PER_CORE]
        in_maps.append({"xin": _pack_x(shard), "w9": w9, "wd": wd, "trig": trig})
    return in_maps


def kernel(x, angles):
    from concourse.bass_utils import run_bass_kernel_spmd

    x = np.asarray(x)
    orig_shape = x.shape
    in_maps = make_in_maps(x, angles)
    nc = _get_nc()
    res = run_bass_kernel_spmd(nc, in_maps, core_ids=list(range(N_CORES)))
    parts = [_unpack_y(res.results[k]["yout"]) for k in range(N_CORES)]
    out = np.concatenate(parts, axis=0).reshape(orig_shape)
    return out.astype(np.float32)


# revision 7
# speedup vs baseline: 10.2771x; 1.1978x over previous
"""Butterfly rotation (10 stages, DIM=1024) on 8 Trainium2 NeuronCores.

Math: the 10-stage butterfly transform is linear.  Stages 0..8 (strides
1..256) mix only within 512-wide blocks: their composite is block-diagonal
with two dense 512x512 blocks, applied on the PE as fp16 matmuls (the
2e-2 error budget admits fp16 end-to-end; measured pipeline error ~1e-3).
Stage 9 (stride 512) pairs chunk cg with cg+4 (128-dim chunks) and is an
elementwise per-partition rotation.

Measured per-[128,1024]-pass engine menu (this deployment): ACT copy from
PSUM 731 ns / with [128,1] scale 1025 ns; DVE fp16 stt 815 ns / TT 635 ns
(SBUF; PSUM-sourced DVE is 1.8 us - avoided); GPSIMD ~8-15 us (unusable);
PE fp16 [128,128,512] matmul 98 ns; DMA floor ~41 us.  PSUM eviction must
therefore ride ACT, stage-9 combines ride DVE in SBUF, and the PE (which
has slack under the DMA roofline) absorbs some chunk pairs entirely as
dense 1024-wide rows of the full 10-stage composite.

Per stage-9 pair (cg, cg+4), three implementations (mix is tuned to
balance engines):
  dense:  PE computes the final outputs via dense rows (16 MM/chunk);
          ACT evicts straight into the output tile.  No DVE work.
  shear:  PE computes stage-0..8 chunks z (8 MM/chunk, with the stage-9
          sign sg = sign(cos th9) folded into the weight rows so the
          residual rotation angle th9' has cos >= 0); ACT evicts z to
          fp16; DVE applies the rotation as 3 shears
          (u += a.v; v += s.u; u += a.v with a = -tan(th9'/2), |a| <= 1).
  acth:   like shear but ACT evicts 4 scaled copies (c9'.z_lo, -s9'.z_hi,
          s9'.z_lo, c9'.z_hi) and DVE just adds pairs (2 TT) - shifts
          work from DVE to ACT.

Device layout (per core, rows sharded 8192/core; pure data parallelism):
  host packs each core's shard dim-major fp16:
  xin[g, p, c*1024 + r] = x[g*1024 + r, c*128 + p]  (g: 8 row-groups of
  1024 rows, c: 8 dim-chunks of 128, p: dim-within-chunk).  Loads ride
  the SP HWDGE ring, stores the ACT ring, ~1 MiB per transfer.  Host
  inverse-permutes and upcasts the fp16 output.
"""

import os
import sys

sys.path.insert(0, "/opt/trn_rl_repo")

# run_bass_kernel_spmd would try to import the (absent) axon NTFF hook if
# BASS_TRACE is set in the environment.
os.environ["BASS_NEVER_TRACE"] = "1"

import numpy as np

DIM = 1024
STAGES = 10
N_CORES = 8
ROWS_PER_CORE = 8192
GROUP_ROWS = 1024
N_GROUPS = ROWS_PER_CORE // GROUP_ROWS  # 8

# stage-9 pair cg -> implementation; tuned so PE/ACT/DVE land together
PAIR_MODES = ["dense", "shear", "shear", "shear"]


def _stage_idx(dim, stage):
    stride = 2**stage
    idx_i = np.arange(dim).reshape(-1, 2 * stride)[:, :stride].ravel()
    idx_j = idx_i + stride
    return idx_i, idx_j


def _butterfly_apply(v, angles, stages):
    """Apply butterfly stages to rows of v (float64, in place) and return v."""
    for s in stages:
        idx_i, idx_j = _stage_idx(v.shape[1], s)
        c = np.cos(angles[s].astype(np.float64))
        sn = np.sin(angles[s].astype(np.float64))
        vi = v[:, idx_i].copy()
        vj = v[:, idx_j].copy()
        v[:, idx_i] = c * vi - sn * vj
        v[:, idx_j] = sn * vi + c * vj
    return v


def _host_tables(angles):
    """w9[k, c*4+t, m] fp16: lhsT for stage-0..8 output chunk c from input
    chunk ci = 4*(c//4) + t, rows pre-scaled by sg9[c % 4] (stage-9 sign).
    wd[k, c*8+t, m] fp16: dense full-composite lhsT, output chunk c from
    input chunk t.
    trig[p, 0..3]=a9[cg], [4..7]=sin th9', [8..11]=cos th9', [12..15]=-sin.
    """
    # _butterfly_apply on eye gives mb[i, j] = M[j, i] (M maps in->out),
    # so lhsT[k, m] = M[c*128+m, ci*128+k] = mb[ci*128+k, c*128+m].
    mb9 = _butterfly_apply(np.eye(DIM, dtype=np.float64), angles, range(9))
    mbA = _butterfly_apply(mb9.copy(), angles, [9])
    mask = np.ones((DIM, DIM), dtype=bool)
    for h in range(2):
        mask[h * 512 : (h + 1) * 512, h * 512 : (h + 1) * 512] = False
    assert abs(mb9[mask]).max() == 0.0

    th9 = angles[9].astype(np.float64)
    sg9, cth, sth, a9 = [], [], [], []
    for cg in range(4):
        th = th9[cg * 128 : (cg + 1) * 128]
        sg = np.where(np.cos(th) >= 0, 1.0, -1.0)
        c9, s9 = sg * np.cos(th), sg * np.sin(th)
        sg9.append(sg)
        cth.append(c9)
        sth.append(s9)
        a9.append(-np.tan(np.arctan2(s9, c9) / 2))
        assert np.abs(a9[-1]).max() <= 1.0 + 1e-12

    w9 = np.empty((128, 32, 128), dtype=np.float16)
    for c in range(8):
        for t in range(4):
            ci = 4 * (c // 4) + t
            blk = mb9[ci * 128 : (ci + 1) * 128, c * 128 : (c + 1) * 128]
            w9[:, c * 4 + t, :] = (blk * sg9[c % 4][None, :]).astype(np.float16)
    wd = np.empty((128, 64, 128), dtype=np.float16)
    for c in range(8):
        for t in range(8):
            wd[:, c * 8 + t, :] = mbA[
                t * 128 : (t + 1) * 128, c * 128 : (c + 1) * 128
            ].astype(np.float16)

    trig = np.empty((128, 16), dtype=np.float32)
    for cg in range(4):
        trig[:, 0 + cg] = a9[cg]
        trig[:, 4 + cg] = sth[cg]
        trig[:, 8 + cg] = cth[cg]
        trig[:, 12 + cg] = -sth[cg]
    return w9, wd, trig


def _pack_x(x_core, n_groups=N_GROUPS):
    # [G*1024, 1024] -> [G, 128, 8192] with xin[g, p, c*1024+r] = x[g*1024+r, c*128+p]
    g = x_core.reshape(n_groups, GROUP_ROWS, 8, 128)
    return np.ascontiguousarray(
        g.transpose(0, 3, 2, 1).reshape(n_groups, 128, 8 * GROUP_ROWS)
    )


# device output slot s holds chunk SLOT_CHUNK[s]: pair (cg, cg+4) lands in
# adjacent slots (2cg, 2cg+1) so each pair's PSUM/eviction is contiguous
SLOT_CHUNK = [0, 4, 1, 5, 2, 6, 3, 7]


def _unpack_y(y_packed, n_groups=N_GROUPS):
    # yout[g, p, slot(c)*1024 + r] = y[g*1024 + r, c*128 + p]
    g = y_packed.reshape(n_groups, 128, 8, GROUP_ROWS)
    g = g.transpose(0, 3, 2, 1)  # [g, r, slot, p]
    chunk_slot = np.argsort(SLOT_CHUNK)  # chunk -> slot
    g = g[:, :, chunk_slot, :]
    return np.ascontiguousarray(g.reshape(n_groups * GROUP_ROWS, DIM))


def _patch_tile_drain():
    """Workaround: this walrus build cannot encode semaphore waits on a
    sequencer Drain/NoOp with >1 wait ("Too many sync wait commands").
    Re-emit the TileContext tail waits as one nop per semaphore."""
    from concourse import mybir, tile
    from concourse.vector_clock import ScopedClock

    if getattr(tile.TileContext, "_drain_patched", False):
        return

    def _drain_and_barrier(self, tick_clock, wait_clock):
        nop_inst = self.nc.sync.nop(nofuse=True)
        wait_clock.add_sem_waits(
            nop_inst.ins, ScopedClock({None: tick_clock.global_clock})
        )
        si = nop_inst.ins.sync_info
        if si is not None and si.on_wait and len(si.on_wait) > 1:
            extra = si.on_wait[1:]
            si.on_wait = si.on_wait[:1]
            for w in extra:
                extra_nop = self.nc.sync.nop(nofuse=True)
                esi = extra_nop.ins.sync_info
                if esi is None:
                    extra_nop.ins.sync_info = mybir.SyncInfo(on_wait=[w], on_update=[])
                else:
                    esi.on_wait = list(esi.on_wait or []) + [w]
        self.nc.sync.drain()
        self.nc.all_engine_barrier()
        assert self.sems is not None
        popped = self.nc._tile_sem_poison_stack.pop()
        assert popped is self._sem_poison
        self.nc.clear_and_free_semaphores(list(self.sems.allocated().values()))
        self.nc.all_engine_barrier()

    tile.TileContext._drain_and_barrier = _drain_and_barrier
    tile.TileContext._drain_patched = True


def _split_multi_waits(nc, limit=1):
    """This walrus build encodes at most `limit` semaphore wait(s) per
    instruction ("Too many sync wait commands").  Hoist excess waits onto
    same-engine NoOps inserted immediately before the instruction."""
    from concourse import mybir

    counter = [0]

    def fresh_nop(engine, waits):
        counter[0] += 1
        nop = mybir.InstNoOp(
            name=f"waitsplit-{counter[0]}",
            engine=engine,
            ins=[],
            outs=[],
            bass_nofuse=True,
            sync_info=mybir.SyncInfo(on_wait=list(waits), on_update=[]),
        )
        nc.register_instruction(nop, overwrite=True)
        return nop

    for fn in nc.m.functions:
        for bb in fn.blocks:
            changed = False
            new = []
            for inst in bb.instructions:
                si = getattr(inst, "sync_info", None)
                if si is not None and si.on_wait and len(si.on_wait) > limit:
                    extra = si.on_wait[: len(si.on_wait) - limit]
                    si.on_wait = si.on_wait[len(si.on_wait) - limit :]
                    for k in range(0, len(extra), limit):
                        new.append(fresh_nop(inst.engine, extra[k : k + limit]))
                    changed = True
                new.append(inst)
            if changed:
                bb.instructions = new
    return nc


def build_bass(n_groups=N_GROUPS, reps=1, pair_modes=None):
    """Build the Bass module for one core processing n_groups row-groups.
    reps>1 repeats the whole pipeline in-NEFF (for timing calibration)."""
    _patch_tile_drain()
    from concourse import bass, mybir, tile

    pair_modes = pair_modes or PAIR_MODES
    f16 = mybir.dt.float16
    f32 = mybir.dt.float32
    nc = bass.Bass("TRN2", target_bir_lowering=False, debug=False)
    xin = nc.dram_tensor("xin", [n_groups, 128, 8192], f16, kind="ExternalInput")
    w9d = nc.dram_tensor("w9", [128, 32, 128], f16, kind="ExternalInput")
    wdd = nc.dram_tensor("wd", [128, 64, 128], f16, kind="ExternalInput")
    trig = nc.dram_tensor("trig", [128, 16], f32, kind="ExternalInput")
    yout = nc.dram_tensor("yout", [n_groups, 128, 8192], f16, kind="ExternalOutput")

    mult = mybir.AluOpType.mult
    add = mybir.AluOpType.add
    copy_fn = mybir.ActivationFunctionType.Copy

    def mm_pair(psum, wtile, cg, nw, xt):
        """Fill a pair's [128, 2048] PSUM tile: chunk cg in cols 0:1024,
        chunk cg+4 in 1024:2048, each accumulated from nw input chunks."""
        for side, c in enumerate((cg, cg + 4)):
            for t in range(nw):
                ci = (c // 4) * 4 + t if nw == 4 else t
                for h in range(2):
                    nc.tensor.matmul(
                        psum[:, side * 1024 + h * 512 : side * 1024 + (h + 1) * 512],
                        wtile[:, c * nw + t, :],
                        xt[:, ci * 1024 + h * 512 : ci * 1024 + (h + 1) * 512],
                        start=(t == 0),
                        stop=(t == nw - 1),
                    )

    with tile.TileContext(nc) as tc:
        with (
            tc.tile_pool(name="wp", bufs=1) as wp,
            tc.tile_pool(name="xp", bufs=3) as xp,
            tc.tile_pool(name="yp", bufs=2) as yp,
            tc.tile_pool(name="ep", bufs=5) as ep,
            tc.tile_pool(name="tp", bufs=4) as tp,
            tc.tile_pool(name="ps", bufs=2, space="PSUM") as psp,
        ):
            w9 = wp.tile([128, 32, 128], f16)
            nc.sync.dma_start(w9[:], w9d.ap()[:])
            wd = wp.tile([128, 64, 128], f16)
            nc.sync.dma_start(wd[:], wdd.ap()[:])
            tg = wp.tile([128, 16], f32)
            nc.sync.dma_start(tg[:], trig.ap()[:])

            for g in [g for _ in range(reps) for g in range(n_groups)]:
                xt = xp.tile([128, 8192], f16)
                nc.sync.dma_start(xt[:, 0:4096], xin.ap()[g][:, 0:4096])
                nc.sync.dma_start(xt[:, 4096:8192], xin.ap()[g][:, 4096:8192])
                yt = yp.tile([128, 8192], f16)
                for half in (0, 1):
                    cgs = (2 * half, 2 * half + 1)
                    ps, ev = {}, {}
                    # PE: both pairs' matmuls
                    for cg in cgs:
                        p = psp.tile([128, 2048], f32, tag="ps")
                        ps[cg] = p
                        if pair_modes[cg] == "dense":
                            mm_pair(p, wd, cg, 8, xt)
                        else:
                            mm_pair(p, w9, cg, 4, xt)
                    # ACT: one FD=2048 eviction per pair
                    for cg in cgs:
                        ysl = yt[:, 2 * cg * 1024 : (2 * cg + 2) * 1024]
                        if pair_modes[cg] == "dense":
                            nc.scalar.copy(ysl, ps[cg][:])
                        else:
                            e = ep.tile([128, 2048], f16, tag="e")
                            nc.scalar.copy(e[:], ps[cg][:])
                            ev[cg] = e
                    # DVE: shear waves interleaved across the half's pairs
                    sh = [cg for cg in cgs if pair_modes[cg] == "shear"]
                    u1 = {}
                    for cg in sh:  # wave 1: u1 = a9.E_hi + E_lo
                        u1[cg] = tp.tile([128, 1024], f16, tag="t", name=f"u1_{cg}")
                        nc.vector.scalar_tensor_tensor(
                            u1[cg][:], ev[cg][:, 1024:2048],
                            tg[:, 0 + cg : 1 + cg], ev[cg][:, 0:1024], mult, add,
                        )
                    for cg in sh:  # wave 2: y_hi = s9'.u1 + E_hi
                        nc.vector.scalar_tensor_tensor(
                            yt[:, (2 * cg + 1) * 1024 : (2 * cg + 2) * 1024],
                            u1[cg][:], tg[:, 4 + cg : 5 + cg],
                            ev[cg][:, 1024:2048], mult, add,
                        )
                    for cg in sh:  # wave 3: y_lo = a9.y_hi + u1
                        nc.vector.scalar_tensor_tensor(
                            yt[:, 2 * cg * 1024 : (2 * cg + 1) * 1024],
                            yt[:, (2 * cg + 1) * 1024 : (2 * cg + 2) * 1024],
                            tg[:, 0 + cg : 1 + cg], u1[cg][:], mult, add,
                        )
                    nc.gpsimd.dma_start(
                        yout.ap()[g][:, half * 4096 : (half + 1) * 4096],
                        yt[:, half * 4096 : (half + 1) * 4096],
                    )
    _split_multi_waits(nc)
    return nc


_CACHE = {}


def _get_nc(n_groups=N_GROUPS):
    if n_groups not in _CACHE:
        _CACHE[n_groups] = build_bass(n_groups)
    return _CACHE[n_groups]


def make_in_maps(x, angles):
    """Pack full inputs into per-core in_maps (list of dicts)."""
    x = np.asarray(x, dtype=np.float32)
    angles = np.asarray(angles, dtype=np.float32)
    w9, wd, trig = _host_tables(angles)
    flat = x.reshape(-1, DIM).astype(np.float16)
    in_maps = []
    for k in range(N_CORES):
        shard = flat[k * ROWS_PER_CORE : (k + 1) * ROWS_PER_CORE]
        in_maps.append({"xin": _pack_x(shard), "w9": w9, "wd": wd, "trig": trig})
    return in_maps


def kernel(x, angles):
    from concourse.bass_utils import run_bass_kernel_spmd

    x = np.asarray(x)
    orig_shape = x.shape
    in_maps = make_in_maps(x, angles)
    nc = _get_nc()
    res = run_bass_kernel_spmd(nc, in_maps, core_ids=list(range(N_CORES)))
    parts = [_unpack_y(res.results[k]["yout"]) for k in range(N_CORES)]
    out = np.concatenate(parts, axis=0).reshape(orig_shape)
    return out.astype(np.float32)
